# revision 5
# baseline (speedup 1.0000x reference)
"""Trainium2 Bass kernel for nn_Anchor3 (segment_reduce): 8-core SPMD.

Per sharding hint: shard the 1M nodes 8 ways. Per core:
  - segment-sum node features into [4096, 64] class tables via SWDGE
    dma_scatter_add (host pre-arranges each shard so every 384-row scatter
    call has unique class ids -> exact accumulation, no CCE RMW races)
  - AllReduce the per-core partial sums across the 8 cores
  - class-level cross-attention (queries sharded 512/core), AllGather the
    class update table
  - per-node output rows via SWDGE dma_gather on the class table
Counts are index metadata of the sharding (host bincount while dealing);
the host feeds 1/(cnt+eps) replicated.
"""
import functools

import numpy as np

import concourse.bass as bass
import concourse.bacc as bacc
import concourse.mybir as mybir
import concourse.tile as tile
from concourse import library_config
from concourse.bass_utils import run_bass_kernel_spmd

N_CORES = 8
NV = 1_000_000
VN = 4096          # classes per branch
E = 64
H = 4
HD = 16
SHARD = NV // N_CORES            # 125000

SC = 384                         # scatter-add call size (ring-safe)
NCH = 352                        # scatter chunks per shard-branch
NSLOT = NCH * SC                 # 135168 dealt slots
PIECE = SC * 44                  # 16896 slots per load piece
NPIECE = NSLOT // PIECE          # 8

GC = 512                         # gather call size (ring-safe)
GSLOT = 131072                   # gather slots (>= SHARD, 16384-aligned)
GPIECE = 16384
NGPIECE = GSLOT // GPIECE        # 8

TBL = 4160                       # 4096 class rows + 64 dummy rows per branch
QCH = VN // N_CORES              # 512 query rows per core
TC = VN // 128                   # 32 key chunks

DT = mybir.dt.float32
I16 = mybir.dt.int16


def _build():
    nc = bacc.Bacc("TRN2", num_swdge_queues=4)

    ins = {}
    for br in ("v", "c"):
        ins[f"sdat_{br}"] = nc.declare_dram_parameter(
            f"sdat_{br}", [128, NSLOT // 128, E], DT, isOutput=False)
        ins[f"sidx_{br}"] = nc.declare_dram_parameter(
            f"sidx_{br}", [128, NSLOT // 16], I16, isOutput=False)
        ins[f"gidx_{br}"] = nc.declare_dram_parameter(
            f"gidx_{br}", [128, GSLOT // 16], I16, isOutput=False)
        ins[f"invrep_{br}"] = nc.declare_dram_parameter(
            f"invrep_{br}", [128, TC, E], DT, isOutput=False)
        ins[f"semq_{br}"] = nc.declare_dram_parameter(
            f"semq_{br}", [E, QCH], DT, isOutput=False)
        ins[f"wqT_{br}"] = nc.declare_dram_parameter(f"wqT_{br}", [E, 128], DT, isOutput=False)
        ins[f"wkT_{br}"] = nc.declare_dram_parameter(f"wkT_{br}", [E, 128], DT, isOutput=False)
        ins[f"wvT_{br}"] = nc.declare_dram_parameter(f"wvT_{br}", [E, E], DT, isOutput=False)
        ins[f"woT_{br}"] = nc.declare_dram_parameter(f"woT_{br}", [E, E], DT, isOutput=False)
        ins[f"bq_{br}"] = nc.declare_dram_parameter(f"bq_{br}", [128, 1], DT, isOutput=False)
        ins[f"bk_{br}"] = nc.declare_dram_parameter(f"bk_{br}", [128, 1], DT, isOutput=False)
        ins[f"bv_{br}"] = nc.declare_dram_parameter(f"bv_{br}", [E, 1], DT, isOutput=False)
        ins[f"bo_{br}"] = nc.declare_dram_parameter(f"bo_{br}", [E, 1], DT, isOutput=False)
    ident = nc.declare_dram_parameter("ident", [128, 128], DT, isOutput=False)
    out_ext = nc.declare_dram_parameter("out", [2, GSLOT, E], DT, isOutput=True)

    acc = nc.dram_tensor("acc", [2 * TBL, E], DT)
    acc_red = nc.dram_tensor("acc_red", [2 * TBL, E], DT, addr_space="Shared")
    fin_own = {br: nc.dram_tensor(f"fin_own_{br}", [QCH, E], DT) for br in ("v", "c")}
    fin_all = {br: nc.dram_tensor(f"fin_all_{br}", [VN, E], DT, addr_space="Shared")
               for br in ("v", "c")}

    rg = [list(range(N_CORES))]
    qn = [0]

    with tile.TileContext(nc) as tc:
        nc.gpsimd.load_library(library_config.mlp)

        # ---- zero accumulator, scatter-add, all-reduce ----
        with tc.tile_pool(name="zp", bufs=1) as zp:
            zt = zp.tile([128, (2 * TBL // 128) * E], DT)
            nc.vector.memset(zt[:], 0.0)
            nc.sync.dma_start(
                out=acc.rearrange("(a p) f -> p a f", p=128),
                in_=zt[:].rearrange("p (a f) -> p a f", f=E),
            )

            with tc.tile_pool(name="scp", bufs=3) as scp:
                for br in ("v", "c"):
                    sdat, sidx = ins[f"sdat_{br}"], ins[f"sidx_{br}"]
                    for pi in range(NPIECE):
                        p0 = pi * PIECE
                        dt_ = scp.tile([128, (PIECE // 128) * E], DT, name="dt")
                        it_ = scp.tile([128, PIECE // 16], I16, name="it")
                        nc.sync.dma_start(
                            out=dt_[:],
                            in_=sdat[:, p0 // 128 : (p0 + PIECE) // 128, :].rearrange(
                                "p s e -> p (s e)"),
                        )
                        nc.sync.dma_start(
                            out=it_[:], in_=sidx[:, p0 // 16 : (p0 + PIECE) // 16])
                        d3 = dt_[:].rearrange("p (s e) -> p s e", e=E)
                        for off in range(0, PIECE, SC):
                            nc.gpsimd.dma_scatter_add(
                                acc[:],
                                d3[:, off // 128 : (off + SC) // 128, :],
                                it_[:, off // 16 : (off + SC) // 16],
                                SC, SC, E,
                                queue_num=qn[0] % 4,
                            )
                            qn[0] += 1

            nc.gpsimd.collective_compute(
                "AllReduce", mybir.AluOpType.add,
                ins=[acc[:]], outs=[acc_red[:]], replica_groups=rg,
            )

        # ---- per-branch: fea, attention, final table ----
        for br in ("v", "c"):
            with tc.tile_pool(name=f"ap_{br}", bufs=1) as ap:
                base = (0 if br == "v" else TBL)
                sums = ap.tile([128, TC, E], DT, name="sums")
                nc.sync.dma_start(
                    out=sums[:],
                    in_=acc_red[base : base + VN, :].rearrange("(a p) f -> p a f", p=128),
                )
                invr = ap.tile([128, TC, E], DT, name="invr")
                nc.sync.dma_start(out=invr[:], in_=ins[f"invrep_{br}"][:])
                fea = ap.tile([128, TC, E], DT, name="fea")
                nc.vector.tensor_tensor(
                    fea[:].rearrange("p a e -> p (a e)"),
                    sums[:].rearrange("p a e -> p (a e)"),
                    invr[:].rearrange("p a e -> p (a e)"),
                    mybir.AluOpType.mult,
                )

                idt = ap.tile([128, 128], DT, name="idt")
                nc.sync.dma_start(out=idt[:], in_=ident[:])

                # transpose fea -> feaT [64, 4096]
                feaT = ap.tile([E, VN], DT, name="feaT")
                with tc.tile_pool(name=f"pT_{br}", bufs=2,
                                  space=bass.MemorySpace.PSUM) as pT:
                    for q in range(4):
                        ptt = pT.tile([E, 1024], DT, name="ptt")
                        for a in range(8):
                            nc.tensor.transpose(
                                ptt[:, a * 128 : (a + 1) * 128],
                                fea[:, q * 8 + a, :], idt[:])
                        nc.vector.tensor_copy(
                            feaT[:, q * 1024 : (q + 1) * 1024], ptt[:])

                # weights
                wq = ap.tile([E, 128], DT, name="wq"); nc.sync.dma_start(out=wq[:], in_=ins[f"wqT_{br}"][:])
                wk = ap.tile([E, 128], DT, name="wk"); nc.sync.dma_start(out=wk[:], in_=ins[f"wkT_{br}"][:])
                wv = ap.tile([E, E], DT, name="wv"); nc.sync.dma_start(out=wv[:], in_=ins[f"wvT_{br}"][:])
                wo = ap.tile([E, E], DT, name="wo"); nc.sync.dma_start(out=wo[:], in_=ins[f"woT_{br}"][:])
                bq = ap.tile([128, 1], DT, name="bq"); nc.sync.dma_start(out=bq[:], in_=ins[f"bq_{br}"][:])
                bk = ap.tile([128, 1], DT, name="bk"); nc.sync.dma_start(out=bk[:], in_=ins[f"bk_{br}"][:])
                bv = ap.tile([E, 1], DT, name="bv"); nc.sync.dma_start(out=bv[:], in_=ins[f"bv_{br}"][:])
                bo = ap.tile([E, 1], DT, name="bo"); nc.sync.dma_start(out=bo[:], in_=ins[f"bo_{br}"][:])
                smq = ap.tile([E, QCH], DT, name="smq")
                nc.sync.dma_start(out=smq[:], in_=ins[f"semq_{br}"][:])

                ktile = ap.tile([128, VN], DT, name="ktile")
                qtile = ap.tile([128, QCH], DT, name="qtile")
                vtile = ap.tile([128, TC, 17 * H], DT, name="vtile")
                with tc.tile_pool(name=f"pP_{br}", bufs=2,
                                  space=bass.MemorySpace.PSUM) as pP:
                    for ch in range(VN // 512):
                        kps = pP.tile([128, 512], DT, name="kps")
                        nc.tensor.matmul(
                            kps[:], wk[:], feaT[:, ch * 512 : (ch + 1) * 512])
                        nc.vector.tensor_scalar_add(
                            ktile[:, ch * 512 : (ch + 1) * 512], kps[:], bk[:])
                    qps = pP.tile([128, QCH], DT, name="kps")
                    nc.tensor.matmul(qps[:], wq[:], smq[:])
                    nc.vector.tensor_scalar_add(qtile[:], qps[:], bq[:])

                    for h in range(H):
                        nc.vector.memset(vtile[:, :, 17 * h : 17 * h + 1], 1.0)
                    for a in range(TC):
                        vps = pP.tile([128, E], DT, name="vps")
                        nc.tensor.matmul(
                            vps[:], feaT[:, a * 128 : (a + 1) * 128], wv[:])
                        nc.vector.tensor_copy(
                            vtile[:, a, :].rearrange("p (h d) -> p h d", d=17)[:, :, 1:17],
                            vps[:].rearrange("p (h d) -> p h d", d=16),
                        )

                # attention: stream over key chunks, accumulate AV + sumexp
                attnT = ap.tile([E, QCH], DT, name="attnT")
                with tc.tile_pool(name=f"pA_{br}", bufs=1,
                                  space=bass.MemorySpace.PSUM) as pA:
                    avps = [pA.tile([17, QCH], DT, name=f"avps{h}")
                            for h in range(H)]
                    with tc.tile_pool(name=f"pS_{br}", bufs=4,
                                      space=bass.MemorySpace.PSUM) as pS, \
                         tc.tile_pool(name=f"eS_{br}", bufs=4) as eS:
                        for a in range(TC):
                            for h in range(H):
                                scp_t = pS.tile([128, QCH], DT, name="scp")
                                nc.tensor.matmul(
                                    scp_t[:],
                                    ktile[32 * h : 32 * h + 32,
                                          a * 128 : (a + 1) * 128],
                                    qtile[32 * h : 32 * h + 32, :],
                                    tile_position=(32 * h, 0),
                                )
                                ex = eS.tile([128, QCH], DT, name="ex")
                                nc.scalar.activation(
                                    ex[:], scp_t[:],
                                    mybir.ActivationFunctionType.Exp)
                                nc.tensor.matmul(
                                    avps[h][:],
                                    vtile[:, a, 17 * h : 17 * h + 17],
                                    ex[:],
                                    start=(a == 0), stop=(a == TC - 1),
                                    skip_group_check=True,
                                )

                    with tc.tile_pool(name=f"pN_{br}", bufs=1,
                                      space=bass.MemorySpace.PSUM) as pN, \
                         tc.tile_pool(name=f"eN_{br}", bufs=1) as eN:
                        one17 = eN.tile([1, 17], DT, name="one17")
                        nc.vector.memset(one17[:], 1.0)
                        for h in range(H):
                            rec = eN.tile([1, QCH], DT, name="rec", bufs=2)
                            nc.vector.reciprocal(rec[:], avps[h][0:1, :])
                            rbc = pN.tile([17, QCH], DT, name="rbc", bufs=2)
                            nc.tensor.matmul(rbc[:], one17[:], rec[:])
                            rbs = eN.tile([17, QCH], DT, name="rbs", bufs=2)
                            nc.vector.tensor_copy(rbs[:], rbc[:])
                            at_ = eN.tile([17, QCH], DT, name="at", bufs=2)
                            nc.vector.tensor_tensor(
                                at_[:], avps[h][0:17, :], rbs[:],
                                mybir.AluOpType.mult)
                            nc.sync.dma_start(
                                out=attnT[16 * h : 16 * h + 16, :],
                                in_=at_[1:17, :])

                # + bv (softmax rows sum to 1), out-proj, transpose, stage
                nc.vector.tensor_scalar_add(attnT[:], attnT[:], bv[:])
                frow = ap.tile([128, QCH // 128, E], DT, name="frow")
                with tc.tile_pool(name=f"pF_{br}", bufs=2,
                                  space=bass.MemorySpace.PSUM) as pF:
                    fps = pF.tile([E, QCH], DT, name="fps", bufs=1)
                    nc.tensor.matmul(fps[:], wo[:], attnT[:])
                    fT = ap.tile([E, QCH], DT, name="fT")
                    nc.vector.tensor_scalar_add(fT[:], fps[:], bo[:])
                    for i in range(QCH // 128):
                        tp = pF.tile([128, E], DT, name="tp")
                        nc.tensor.transpose(
                            tp[:], fT[:, i * 128 : (i + 1) * 128], idt[0:E, 0:E])
                        nc.vector.tensor_copy(frow[:, i, :], tp[:])
                nc.sync.dma_start(
                    out=fin_own[br].rearrange("(i p) e -> p i e", p=128),
                    in_=frow[:])

                nc.gpsimd.collective_compute(
                    "AllGather", mybir.AluOpType.bypass,
                    ins=[fin_own[br][:]], outs=[fin_all[br][:]],
                    replica_groups=rg,
                )

        # ---- output gather, both branches ----
        with tc.tile_pool(name="gp", bufs=3) as gp:
            for br_i, br in enumerate(("v", "c")):
                gidx = ins[f"gidx_{br}"]
                for pi in range(NGPIECE):
                    p0 = pi * GPIECE
                    it_ = gp.tile([128, GPIECE // 16], I16, name="git")
                    nc.sync.dma_start(
                        out=it_[:], in_=gidx[:, p0 // 16 : (p0 + GPIECE) // 16])
                    gt_ = gp.tile([128, (GPIECE // 128) * E], DT, name="gt")
                    g3 = gt_[:].rearrange("p (s e) -> p s e", e=E)
                    for off in range(0, GPIECE, GC):
                        nc.gpsimd.dma_gather(
                            g3[:, off // 128 : (off + GC) // 128, :],
                            fin_all[br][:],
                            it_[:, off // 16 : (off + GC) // 16],
                            GC, GC, E,
                            queue_num=qn[0] % 4,
                        )
                        qn[0] += 1
                    nc.sync.dma_start(
                        out=out_ext[br_i, p0 : p0 + GPIECE, :].rearrange(
                            "(s p) e -> p s e", p=128),
                        in_=g3[:],
                    )
    nc.compile()
    return nc


@functools.cache
def _compiled():
    return _build()


# ------------------------- host-side preparation -------------------------

def _deal(cls_shard: np.ndarray, salt: int):
    """Arrange shard rows into NCH chunks of SC slots, each chunk free of
    duplicate class ids. Returns int64 [NSLOT] of source row ids (-1 = pad)."""
    n = cls_shard.shape[0]
    order = np.argsort(cls_shard, kind="stable")
    sc_ = cls_shard[order].astype(np.int64)
    first = np.r_[True, sc_[1:] != sc_[:-1]]
    grp = np.cumsum(first) - 1
    gstart = np.flatnonzero(first)
    rank = np.arange(n) - gstart[grp]
    hsh = (sc_ * 2654435761 + salt) % NCH
    chunk = (rank + hsh) % NCH
    o2 = np.argsort(chunk, kind="stable")
    csrt = chunk[o2]
    tok = order[o2]
    clss = sc_[o2]
    starts = np.searchsorted(csrt, np.arange(NCH))
    pos = np.arange(n) - starts[csrt]
    keep = pos < SC
    slots = np.full(NSLOT, -1, np.int64)
    slots[csrt[keep] * SC + pos[keep]] = tok[keep]
    sp_tok, sp_cls = tok[~keep], clss[~keep]
    if sp_tok.size:
        has = np.zeros((NCH, VN), bool)
        has[csrt[keep], clss[keep]] = True
        load = np.minimum(np.diff(np.r_[starts, n]), SC)
        k0 = 0
        for t, c in zip(sp_tok, sp_cls):
            for dk in range(NCH):
                k = (k0 + dk) % NCH
                if load[k] < SC and not has[k, c]:
                    slots[k * SC + load[k]] = t
                    load[k] += 1
                    has[k, c] = True
                    k0 = k + 1
                    break
            else:
                raise RuntimeError("deal failed")
    return slots


def _wrap_rows(rows: np.ndarray) -> np.ndarray:
    n, e = rows.shape
    return np.ascontiguousarray(rows.reshape(n // 128, 128, e).transpose(1, 0, 2))


def _wrap_idx(idx: np.ndarray) -> np.ndarray:
    n = idx.shape[0]
    w = np.ascontiguousarray(idx.reshape(n // 16, 16).T).astype(np.int16)
    return np.tile(w, (8, 1))


def _branch_weights(in_w, in_b, out_w, out_b):
    in_w = np.asarray(in_w, np.float32)
    in_b = np.asarray(in_b, np.float32)
    wq, wk, wv = in_w[:E], in_w[E:2 * E], in_w[2 * E:]
    bq, bk, bv = in_b[:E], in_b[E:2 * E], in_b[2 * E:]
    scale = np.float32(1.0 / np.sqrt(HD))
    wqT_pad = np.zeros((E, 128), np.float32)
    wkT_pad = np.zeros((E, 128), np.float32)
    bq_pad = np.zeros((128, 1), np.float32)
    bk_pad = np.zeros((128, 1), np.float32)
    for h in range(H):
        for j in range(HD):
            wqT_pad[:, 32 * h + j] = wq[HD * h + j] * scale
            wkT_pad[:, 32 * h + j] = wk[HD * h + j]
            bq_pad[32 * h + j, 0] = bq[HD * h + j] * scale
            bk_pad[32 * h + j, 0] = bk[HD * h + j]
    return {
        "wqT": wqT_pad, "wkT": wkT_pad,
        "wvT": np.ascontiguousarray(wv.T),
        "woT": np.ascontiguousarray(np.asarray(out_w, np.float32).T),
        "bq": bq_pad, "bk": bk_pad,
        "bv": bv.reshape(E, 1).astype(np.float32),
        "bo": np.asarray(out_b, np.float32).reshape(E, 1),
    }


def kernel(v_s, c_s, v_sem, c_sem, v_class, c_class,
           v_in_w, v_in_b, v_out_w, v_out_b,
           c_in_w, c_in_b, c_out_w, c_out_b):
    in_maps = _make_in_maps(v_s, c_s, v_sem, c_sem, v_class, c_class,
                            v_in_w, v_in_b, v_out_w, v_out_b,
                            c_in_w, c_in_b, c_out_w, c_out_b)
    nc = _compiled()
    res = run_bass_kernel_spmd(nc, in_maps, core_ids=list(range(N_CORES)))
    v_out = np.empty((NV, E), np.float32)
    c_out = np.empty((NV, E), np.float32)
    for core in range(N_CORES):
        o = res.results[core]["out"].reshape(2, GSLOT, E)
        v_out[core * SHARD : (core + 1) * SHARD] = o[0, :SHARD]
        c_out[core * SHARD : (core + 1) * SHARD] = o[1, :SHARD]
    return v_out, c_out


# exposed for test.py timing
def prepare_in_maps(inputs):
    """Return (nc, in_maps) without running."""
    import inspect
    sig = ["v_s", "c_s", "v_sem", "c_sem", "v_class", "c_class",
           "v_in_w", "v_in_b", "v_out_w", "v_out_b",
           "c_in_w", "c_in_b", "c_out_w", "c_out_b"]
    return _compiled(), _make_in_maps(**{k: inputs[k] for k in sig})


def _make_in_maps(v_s, c_s, v_sem, c_sem, v_class, c_class,
                  v_in_w, v_in_b, v_out_w, v_out_b,
                  c_in_w, c_in_b, c_out_w, c_out_b):
    # duplicate of kernel()'s prep, returning in_maps
    v_s = np.asarray(v_s, np.float32); c_s = np.asarray(c_s, np.float32)
    v_class = np.asarray(v_class, np.int32); c_class = np.asarray(c_class, np.int32)
    v_semT = np.ascontiguousarray(np.asarray(v_sem, np.float32).T)
    c_semT = np.ascontiguousarray(np.asarray(c_sem, np.float32).T)
    wts = {"v": _branch_weights(v_in_w, v_in_b, v_out_w, v_out_b),
           "c": _branch_weights(c_in_w, c_in_b, c_out_w, c_out_b)}
    ident = np.eye(128, dtype=np.float32)
    invrep = {}
    for br, cls in (("v", v_class), ("c", c_class)):
        cnt = np.bincount(cls, minlength=VN).astype(np.float32)
        inv = (1.0 / (cnt + 1e-8)).astype(np.float32)
        pc = np.ascontiguousarray(inv.reshape(TC, 128).T)
        invrep[br] = np.ascontiguousarray(
            np.broadcast_to(pc[:, :, None], (128, TC, E))).astype(np.float32)
    in_maps = []
    for core in range(N_CORES):
        b0 = core * SHARD
        m = {"ident": ident}
        for br, s_all, cls_all, semT in (
            ("v", v_s, v_class, v_semT), ("c", c_s, c_class, c_semT)):
            cls = cls_all[b0 : b0 + SHARD]
            slots = _deal(cls, salt=1000 * core + (0 if br == "v" else 1))
            real = slots >= 0
            sdat = np.zeros((NSLOT, E), np.float32)
            sdat[real] = s_all[b0 : b0 + SHARD][slots[real]]
            base = 0 if br == "v" else TBL
            sidx = np.zeros(NSLOT, np.int64)
            sidx[real] = base + cls[slots[real]]
            dm = ~real
            sidx[dm] = base + VN + (np.flatnonzero(dm) % 64)
            gidx = np.zeros(GSLOT, np.int64)
            gidx[:SHARD] = cls
            m[f"sdat_{br}"] = _wrap_rows(sdat)
            m[f"sidx_{br}"] = _wrap_idx(sidx)
            m[f"gidx_{br}"] = _wrap_idx(gidx)
            m[f"invrep_{br}"] = invrep[br]
            m[f"semq_{br}"] = np.ascontiguousarray(
                semT[:, core * QCH : (core + 1) * QCH])
            for k, vv in wts[br].items():
                m[f"{k}_{br}"] = vv
        in_maps.append(m)
    return in_maps


# revision 11
# speedup vs baseline: 1.0027x; 1.0027x over previous
"""Trainium2 Bass kernel for nn_Anchor3 (segment_reduce): 8-core SPMD.

Per sharding hint: shard the 1M nodes 8 ways. Per core:
  - segment-sum node features into [4096, 64] class tables via SWDGE
    dma_scatter_add (host pre-arranges each shard so every 384-row scatter
    call has unique class ids -> exact accumulation, no CCE RMW races)
  - AllReduce the per-core partial sums across the 8 cores
  - class-level cross-attention (queries sharded 512/core), AllGather the
    class update table
  - per-node output rows via SWDGE dma_gather on the class table
Counts are index metadata of the sharding (host bincount while dealing);
the host feeds 1/(cnt+eps) replicated.
"""
import functools
import os

import numpy as np

import concourse.bass as bass
import concourse.bacc as bacc
import concourse.mybir as mybir
import concourse.tile as tile
from concourse import library_config
from concourse.bass_utils import run_bass_kernel_spmd

N_CORES = 8
NV = 1_000_000
VN = 4096          # classes per branch
E = 64
H = 4
HD = 16
SHARD = NV // N_CORES            # 125000

SC = 384                         # scatter-add call size (ring-safe)
NCH = 352                        # scatter chunks per shard-branch
NSLOT = NCH * SC                 # 135168 dealt slots
PIECE = SC * 44                  # 16896 slots per load piece
NPIECE = NSLOT // PIECE          # 8

GC = 512                         # gather call size (ring-safe)
GSLOT = 131072                   # gather slots (>= SHARD, 16384-aligned)
GPIECE = 16384
NGPIECE = GSLOT // GPIECE        # 8

TBL = 4160                       # 4096 class rows + 64 dummy rows per branch
QCH = VN // N_CORES              # 512 query rows per core
TC = VN // 128                   # 32 key chunks

DT = mybir.dt.float32
I16 = mybir.dt.int16


def _build():
    skip_sc = bool(os.environ.get("KSKIP_SCATTER"))
    skip_ga = bool(os.environ.get("KSKIP_GATHER"))
    skip_at = bool(os.environ.get("KSKIP_ATTN"))
    skip_br = bool(os.environ.get("KSKIP_BRANCH"))
    skip_ar = bool(os.environ.get("KSKIP_AR"))
    skip_ld = bool(os.environ.get("KSKIP_LOADS"))
    nc = bacc.Bacc("TRN2", num_swdge_queues=4)

    ins = {}
    for br in ("v", "c"):
        ins[f"sdat_{br}"] = nc.declare_dram_parameter(
            f"sdat_{br}", [128, NSLOT // 128, E], DT, isOutput=False)
        ins[f"sidx_{br}"] = nc.declare_dram_parameter(
            f"sidx_{br}", [128, NSLOT // 16], I16, isOutput=False)
        ins[f"gidx_{br}"] = nc.declare_dram_parameter(
            f"gidx_{br}", [128, GSLOT // 16], I16, isOutput=False)
        ins[f"invrep_{br}"] = nc.declare_dram_parameter(
            f"invrep_{br}", [128, TC, E], DT, isOutput=False)
        ins[f"semq_{br}"] = nc.declare_dram_parameter(
            f"semq_{br}", [E, QCH], DT, isOutput=False)
        ins[f"wqT_{br}"] = nc.declare_dram_parameter(f"wqT_{br}", [E, 128], DT, isOutput=False)
        ins[f"wkT_{br}"] = nc.declare_dram_parameter(f"wkT_{br}", [E, 128], DT, isOutput=False)
        ins[f"wvT_{br}"] = nc.declare_dram_parameter(f"wvT_{br}", [E, E], DT, isOutput=False)
        ins[f"woT_{br}"] = nc.declare_dram_parameter(f"woT_{br}", [E, E], DT, isOutput=False)
        ins[f"bq_{br}"] = nc.declare_dram_parameter(f"bq_{br}", [128, 1], DT, isOutput=False)
        ins[f"bk_{br}"] = nc.declare_dram_parameter(f"bk_{br}", [128, 1], DT, isOutput=False)
        ins[f"bv_{br}"] = nc.declare_dram_parameter(f"bv_{br}", [E, 1], DT, isOutput=False)
        ins[f"bo_{br}"] = nc.declare_dram_parameter(f"bo_{br}", [E, 1], DT, isOutput=False)
    ident = nc.declare_dram_parameter("ident", [128, 128], DT, isOutput=False)
    out_ext = nc.declare_dram_parameter("out", [2, GSLOT, E], DT, isOutput=True)

    acc = nc.dram_tensor("acc", [2 * TBL, E], DT)
    acc_red = nc.dram_tensor("acc_red", [2 * TBL, E], DT, addr_space="Shared")
    fin_own = nc.dram_tensor("fin_own", [2 * QCH, E], DT)
    fin_all = nc.dram_tensor("fin_all", [2 * VN, E], DT, addr_space="Shared")

    rg = [list(range(N_CORES))]
    qn = [0]

    with tile.TileContext(nc) as tc:
        nc.gpsimd.load_library(library_config.mlp)

        # ---- zero accumulator, scatter-add, all-reduce ----
        with tc.tile_pool(name="zp", bufs=1) as zp:
            zt = zp.tile([128, (2 * TBL // 128) * E], DT)
            nc.vector.memset(zt[:], 0.0)
            nc.sync.dma_start(
                out=acc.rearrange("(a p) f -> p a f", p=128),
                in_=zt[:].rearrange("p (a f) -> p a f", f=E),
            )

            with tc.tile_pool(name="scp", bufs=4) as scp:
                for pi in ([] if skip_ld else range(NPIECE)):
                    for br in ("v", "c"):
                        sdat, sidx = ins[f"sdat_{br}"], ins[f"sidx_{br}"]
                        base = (0 if br == "v" else TBL)
                        p0 = pi * PIECE
                        dt_ = scp.tile([128, (PIECE // 128) * E], DT, name="dt")
                        it_ = scp.tile([128, PIECE // 16], I16, name="it")
                        nc.sync.dma_start(
                            out=dt_[:],
                            in_=sdat[:, p0 // 128 : (p0 + PIECE) // 128, :].rearrange(
                                "p s e -> p (s e)"),
                        )
                        nc.sync.dma_start(
                            out=it_[:], in_=sidx[:, p0 // 16 : (p0 + PIECE) // 16])
                        d3 = dt_[:].rearrange("p (s e) -> p s e", e=E)
                        for off in ([] if skip_sc else range(0, PIECE, SC)):
                            nc.gpsimd.dma_scatter_add(
                                acc[base : base + TBL, :],
                                d3[:, off // 128 : (off + SC) // 128, :],
                                it_[:, off // 16 : (off + SC) // 16],
                                SC, SC, E,
                                queue_num=qn[0] % 4,
                            )
                            qn[0] += 1

            if not skip_ar:
                nc.gpsimd.collective_compute(
                    "AllReduce", mybir.AluOpType.add,
                    ins=[acc[:]], outs=[acc_red[:]], replica_groups=rg,
                )
            else:
                nc.sync.dma_start(out=acc_red[:1024, :], in_=acc[:1024, :])

        # ---- per-branch: fea, attention, final table ----
        for br in ("v", "c"):
            if skip_br:
                with tc.tile_pool(name=f"sb_{br}", bufs=1) as sb:
                    frow0 = sb.tile([128, QCH // 128, E], DT, name="frow0")
                    nc.vector.memset(frow0[:], 0.0)
                    fo = (0 if br == "v" else QCH)
                    nc.sync.dma_start(
                        out=fin_own[fo : fo + QCH, :].rearrange(
                            "(i p) e -> p i e", p=128),
                        in_=frow0[:])
                continue
            with tc.tile_pool(name=f"ap_{br}", bufs=1) as ap:
                base = (0 if br == "v" else TBL)
                sums = ap.tile([128, TC, E], DT, name="sums")
                nc.sync.dma_start(
                    out=sums[:],
                    in_=acc_red[base : base + VN, :].rearrange("(a p) f -> p a f", p=128),
                )
                invr = ap.tile([128, TC, E], DT, name="invr")
                nc.sync.dma_start(out=invr[:], in_=ins[f"invrep_{br}"][:])
                fea = ap.tile([128, TC, E], DT, name="fea")
                nc.vector.tensor_tensor(
                    fea[:].rearrange("p a e -> p (a e)"),
                    sums[:].rearrange("p a e -> p (a e)"),
                    invr[:].rearrange("p a e -> p (a e)"),
                    mybir.AluOpType.mult,
                )

                idt = ap.tile([128, 128], DT, name="idt")
                nc.sync.dma_start(out=idt[:], in_=ident[:])

                # transpose fea -> feaT [64, 4096]
                feaT = ap.tile([E, VN], DT, name="feaT")
                with tc.tile_pool(name=f"pT_{br}", bufs=2,
                                  space=bass.MemorySpace.PSUM) as pT:
                    for q in range(4):
                        ptt = pT.tile([E, 1024], DT, name="ptt")
                        for a in range(8):
                            nc.tensor.transpose(
                                ptt[:, a * 128 : (a + 1) * 128],
                                fea[:, q * 8 + a, :], idt[:])
                        nc.vector.tensor_copy(
                            feaT[:, q * 1024 : (q + 1) * 1024], ptt[:])

                # weights
                wq = ap.tile([E, 128], DT, name="wq"); nc.sync.dma_start(out=wq[:], in_=ins[f"wqT_{br}"][:])
                wk = ap.tile([E, 128], DT, name="wk"); nc.sync.dma_start(out=wk[:], in_=ins[f"wkT_{br}"][:])
                wv = ap.tile([E, E], DT, name="wv"); nc.sync.dma_start(out=wv[:], in_=ins[f"wvT_{br}"][:])
                wo = ap.tile([E, E], DT, name="wo"); nc.sync.dma_start(out=wo[:], in_=ins[f"woT_{br}"][:])
                bq = ap.tile([128, 1], DT, name="bq"); nc.sync.dma_start(out=bq[:], in_=ins[f"bq_{br}"][:])
                bk = ap.tile([128, 1], DT, name="bk"); nc.sync.dma_start(out=bk[:], in_=ins[f"bk_{br}"][:])
                bv = ap.tile([E, 1], DT, name="bv"); nc.sync.dma_start(out=bv[:], in_=ins[f"bv_{br}"][:])
                bo = ap.tile([E, 1], DT, name="bo"); nc.sync.dma_start(out=bo[:], in_=ins[f"bo_{br}"][:])
                smq = ap.tile([E, QCH], DT, name="smq")
                nc.sync.dma_start(out=smq[:], in_=ins[f"semq_{br}"][:])

                ktile = ap.tile([128, VN], DT, name="ktile")
                qtile = ap.tile([128, QCH], DT, name="qtile")
                vtile = ap.tile([128, TC, 17 * H], DT, name="vtile")
                with tc.tile_pool(name=f"pP_{br}", bufs=2,
                                  space=bass.MemorySpace.PSUM) as pP:
                    for ch in range(VN // 512):
                        kps = pP.tile([128, 512], DT, name="kps")
                        nc.tensor.matmul(
                            kps[:], wk[:], feaT[:, ch * 512 : (ch + 1) * 512])
                        nc.vector.tensor_scalar_add(
                            ktile[:, ch * 512 : (ch + 1) * 512], kps[:], bk[:])
                    qps = pP.tile([128, QCH], DT, name="kps")
                    nc.tensor.matmul(qps[:], wq[:], smq[:])
                    nc.vector.tensor_scalar_add(qtile[:], qps[:], bq[:])

                    for h in range(H):
                        nc.vector.memset(vtile[:, :, 17 * h : 17 * h + 1], 1.0)
                    for a in range(TC):
                        vps = pP.tile([128, E], DT, name="vps")
                        nc.tensor.matmul(
                            vps[:], feaT[:, a * 128 : (a + 1) * 128], wv[:])
                        nc.vector.tensor_copy(
                            vtile[:, a, :].rearrange("p (h d) -> p h d", d=17)[:, :, 1:17],
                            vps[:].rearrange("p (h d) -> p h d", d=16),
                        )

                # attention: stream over key chunks, accumulate AV + sumexp
                attnT = ap.tile([E, QCH], DT, name="attnT")
                with tc.tile_pool(name=f"pA_{br}", bufs=1,
                                  space=bass.MemorySpace.PSUM) as pA:
                    avps = [pA.tile([17, QCH], DT, name=f"avps{h}")
                            for h in range(H)]
                    with tc.tile_pool(name=f"pS_{br}", bufs=4,
                                      space=bass.MemorySpace.PSUM) as pS, \
                         tc.tile_pool(name=f"eS_{br}", bufs=4) as eS:
                        for a in ([0] if skip_at else range(TC)):
                            for h in range(H):
                                scp_t = pS.tile([128, QCH], DT, name="scp")
                                nc.tensor.matmul(
                                    scp_t[:],
                                    ktile[32 * h : 32 * h + 32,
                                          a * 128 : (a + 1) * 128],
                                    qtile[32 * h : 32 * h + 32, :],
                                    tile_position=(32 * h, 0),
                                )
                                ex = eS.tile([128, QCH], DT, name="ex")
                                nc.scalar.activation(
                                    ex[:], scp_t[:],
                                    mybir.ActivationFunctionType.Exp)
                                nc.tensor.matmul(
                                    avps[h][:],
                                    vtile[:, a, 17 * h : 17 * h + 17],
                                    ex[:],
                                    start=(a == 0),
                                    stop=(a == TC - 1 or skip_at),
                                    skip_group_check=True,
                                )

                    with tc.tile_pool(name=f"pN_{br}", bufs=1,
                                      space=bass.MemorySpace.PSUM) as pN, \
                         tc.tile_pool(name=f"eN_{br}", bufs=1) as eN:
                        one17 = eN.tile([1, 17], DT, name="one17")
                        nc.vector.memset(one17[:], 1.0)
                        for h in range(H):
                            rec = eN.tile([1, QCH], DT, name="rec", bufs=2)
                            nc.vector.reciprocal(rec[:], avps[h][0:1, :])
                            rbc = pN.tile([17, QCH], DT, name="rbc", bufs=2)
                            nc.tensor.matmul(rbc[:], one17[:], rec[:])
                            rbs = eN.tile([17, QCH], DT, name="rbs", bufs=2)
                            nc.vector.tensor_copy(rbs[:], rbc[:])
                            at_ = eN.tile([17, QCH], DT, name="at", bufs=2)
                            nc.vector.tensor_tensor(
                                at_[:], avps[h][0:17, :], rbs[:],
                                mybir.AluOpType.mult)
                            nc.sync.dma_start(
                                out=attnT[16 * h : 16 * h + 16, :],
                                in_=at_[1:17, :])

                # + bv (softmax rows sum to 1), out-proj, transpose, stage
                nc.vector.tensor_scalar_add(attnT[:], attnT[:], bv[:])
                frow = ap.tile([128, QCH // 128, E], DT, name="frow")
                with tc.tile_pool(name=f"pF_{br}", bufs=2,
                                  space=bass.MemorySpace.PSUM) as pF:
                    fps = pF.tile([E, QCH], DT, name="fps", bufs=1)
                    nc.tensor.matmul(fps[:], wo[:], attnT[:])
                    fT = ap.tile([E, QCH], DT, name="fT")
                    nc.vector.tensor_scalar_add(fT[:], fps[:], bo[:])
                    for i in range(QCH // 128):
                        tp = pF.tile([128, E], DT, name="tp")
                        nc.tensor.transpose(
                            tp[:], fT[:, i * 128 : (i + 1) * 128], idt[0:E, 0:E])
                        nc.vector.tensor_copy(frow[:, i, :], tp[:])
                fo = (0 if br == "v" else QCH)
                nc.sync.dma_start(
                    out=fin_own[fo : fo + QCH, :].rearrange(
                        "(i p) e -> p i e", p=128),
                    in_=frow[:])

        nc.gpsimd.collective_compute(
            "AllGather", mybir.AluOpType.bypass,
            ins=[fin_own[:]], outs=[fin_all[:]], replica_groups=rg,
        )

        # ---- output gather, both branches ----
        with tc.tile_pool(name="gp", bufs=3) as gp:
            for br_i, br in enumerate(("v", "c")):
                gidx = ins[f"gidx_{br}"]
                for pi in range(NGPIECE):
                    p0 = pi * GPIECE
                    it_ = gp.tile([128, GPIECE // 16], I16, name="git")
                    nc.sync.dma_start(
                        out=it_[:], in_=gidx[:, p0 // 16 : (p0 + GPIECE) // 16])
                    gt_ = gp.tile([128, (GPIECE // 128) * E], DT, name="gt")
                    g3 = gt_[:].rearrange("p (s e) -> p s e", e=E)
                    if skip_ga:
                        nc.vector.memset(gt_[:], 0.0)
                    for off in ([] if skip_ga else range(0, GPIECE, GC)):
                        nc.gpsimd.dma_gather(
                            g3[:, off // 128 : (off + GC) // 128, :],
                            fin_all[:],
                            it_[:, off // 16 : (off + GC) // 16],
                            GC, GC, E,
                            queue_num=qn[0] % 4,
                        )
                        qn[0] += 1
                    nc.sync.dma_start(
                        out=out_ext[br_i, p0 : p0 + GPIECE, :].rearrange(
                            "(s p) e -> p s e", p=128),
                        in_=g3[:],
                    )
    nc.compile()
    return nc


@functools.cache
def _compiled():
    return _build()


# ------------------------- host-side preparation -------------------------

def _deal(cls_shard: np.ndarray, salt: int):
    """Arrange shard rows into NCH chunks of SC slots, each chunk free of
    duplicate class ids. Returns int64 [NSLOT] of source row ids (-1 = pad)."""
    n = cls_shard.shape[0]
    order = np.argsort(cls_shard, kind="stable")
    sc_ = cls_shard[order].astype(np.int64)
    first = np.r_[True, sc_[1:] != sc_[:-1]]
    grp = np.cumsum(first) - 1
    gstart = np.flatnonzero(first)
    rank = np.arange(n) - gstart[grp]
    hsh = (sc_ * 2654435761 + salt) % NCH
    chunk = (rank + hsh) % NCH
    o2 = np.argsort(chunk, kind="stable")
    csrt = chunk[o2]
    tok = order[o2]
    clss = sc_[o2]
    starts = np.searchsorted(csrt, np.arange(NCH))
    pos = np.arange(n) - starts[csrt]
    keep = pos < SC
    slots = np.full(NSLOT, -1, np.int64)
    slots[csrt[keep] * SC + pos[keep]] = tok[keep]
    sp_tok, sp_cls = tok[~keep], clss[~keep]
    if sp_tok.size:
        has = np.zeros((NCH, VN), bool)
        has[csrt[keep], clss[keep]] = True
        load = np.minimum(np.diff(np.r_[starts, n]), SC)
        k0 = 0
        for t, c in zip(sp_tok, sp_cls):
            for dk in range(NCH):
                k = (k0 + dk) % NCH
                if load[k] < SC and not has[k, c]:
                    slots[k * SC + load[k]] = t
                    load[k] += 1
                    has[k, c] = True
                    k0 = k + 1
                    break
            else:
                raise RuntimeError("deal failed")
    return slots


def _wrap_rows(rows: np.ndarray) -> np.ndarray:
    n, e = rows.shape
    return np.ascontiguousarray(rows.reshape(n // 128, 128, e).transpose(1, 0, 2))


def _wrap_idx(idx: np.ndarray) -> np.ndarray:
    n = idx.shape[0]
    w = np.ascontiguousarray(idx.reshape(n // 16, 16).T).astype(np.int16)
    return np.tile(w, (8, 1))


def _branch_weights(in_w, in_b, out_w, out_b):
    in_w = np.asarray(in_w, np.float32)
    in_b = np.asarray(in_b, np.float32)
    wq, wk, wv = in_w[:E], in_w[E:2 * E], in_w[2 * E:]
    bq, bk, bv = in_b[:E], in_b[E:2 * E], in_b[2 * E:]
    scale = np.float32(1.0 / np.sqrt(HD))
    wqT_pad = np.zeros((E, 128), np.float32)
    wkT_pad = np.zeros((E, 128), np.float32)
    bq_pad = np.zeros((128, 1), np.float32)
    bk_pad = np.zeros((128, 1), np.float32)
    for h in range(H):
        for j in range(HD):
            wqT_pad[:, 32 * h + j] = wq[HD * h + j] * scale
            wkT_pad[:, 32 * h + j] = wk[HD * h + j]
            bq_pad[32 * h + j, 0] = bq[HD * h + j] * scale
            bk_pad[32 * h + j, 0] = bk[HD * h + j]
    return {
        "wqT": wqT_pad, "wkT": wkT_pad,
        "wvT": np.ascontiguousarray(wv.T),
        "woT": np.ascontiguousarray(np.asarray(out_w, np.float32).T),
        "bq": bq_pad, "bk": bk_pad,
        "bv": bv.reshape(E, 1).astype(np.float32),
        "bo": np.asarray(out_b, np.float32).reshape(E, 1),
    }


def kernel(v_s, c_s, v_sem, c_sem, v_class, c_class,
           v_in_w, v_in_b, v_out_w, v_out_b,
           c_in_w, c_in_b, c_out_w, c_out_b):
    in_maps = _make_in_maps(v_s, c_s, v_sem, c_sem, v_class, c_class,
                            v_in_w, v_in_b, v_out_w, v_out_b,
                            c_in_w, c_in_b, c_out_w, c_out_b)
    nc = _compiled()
    res = run_bass_kernel_spmd(nc, in_maps, core_ids=list(range(N_CORES)))
    v_out = np.empty((NV, E), np.float32)
    c_out = np.empty((NV, E), np.float32)
    for core in range(N_CORES):
        o = res.results[core]["out"].reshape(2, GSLOT, E)
        v_out[core * SHARD : (core + 1) * SHARD] = o[0, :SHARD]
        c_out[core * SHARD : (core + 1) * SHARD] = o[1, :SHARD]
    return v_out, c_out


# exposed for test.py timing
def prepare_in_maps(inputs):
    """Return (nc, in_maps) without running."""
    import inspect
    sig = ["v_s", "c_s", "v_sem", "c_sem", "v_class", "c_class",
           "v_in_w", "v_in_b", "v_out_w", "v_out_b",
           "c_in_w", "c_in_b", "c_out_w", "c_out_b"]
    return _compiled(), _make_in_maps(**{k: inputs[k] for k in sig})


def _make_in_maps(v_s, c_s, v_sem, c_sem, v_class, c_class,
                  v_in_w, v_in_b, v_out_w, v_out_b,
                  c_in_w, c_in_b, c_out_w, c_out_b):
    # duplicate of kernel()'s prep, returning in_maps
    v_s = np.asarray(v_s, np.float32); c_s = np.asarray(c_s, np.float32)
    v_class = np.asarray(v_class, np.int32); c_class = np.asarray(c_class, np.int32)
    v_semT = np.ascontiguousarray(np.asarray(v_sem, np.float32).T)
    c_semT = np.ascontiguousarray(np.asarray(c_sem, np.float32).T)
    wts = {"v": _branch_weights(v_in_w, v_in_b, v_out_w, v_out_b),
           "c": _branch_weights(c_in_w, c_in_b, c_out_w, c_out_b)}
    ident = np.eye(128, dtype=np.float32)
    invrep = {}
    for br, cls in (("v", v_class), ("c", c_class)):
        cnt = np.bincount(cls, minlength=VN).astype(np.float32)
        inv = (1.0 / (cnt + 1e-8)).astype(np.float32)
        pc = np.ascontiguousarray(inv.reshape(TC, 128).T)
        invrep[br] = np.ascontiguousarray(
            np.broadcast_to(pc[:, :, None], (128, TC, E))).astype(np.float32)
    in_maps = []
    for core in range(N_CORES):
        b0 = core * SHARD
        m = {"ident": ident}
        for br, s_all, cls_all, semT in (
            ("v", v_s, v_class, v_semT), ("c", c_s, c_class, c_semT)):
            cls = cls_all[b0 : b0 + SHARD]
            slots = _deal(cls, salt=1000 * core + (0 if br == "v" else 1))
            real = slots >= 0
            sdat = np.zeros((NSLOT, E), np.float32)
            sdat[real] = s_all[b0 : b0 + SHARD][slots[real]]
            sidx = np.zeros(NSLOT, np.int64)
            sidx[real] = cls[slots[real]]
            dm = ~real
            sidx[dm] = VN + (np.flatnonzero(dm) % 64)
            # AG row layout: rank r holds [v queries 512 | c queries 512]
            boff = 0 if br == "v" else QCH
            gidx = np.zeros(GSLOT, np.int64)
            gidx[:SHARD] = (cls // QCH) * (2 * QCH) + boff + (cls % QCH)
            m[f"sdat_{br}"] = _wrap_rows(sdat)
            m[f"sidx_{br}"] = _wrap_idx(sidx)
            m[f"gidx_{br}"] = _wrap_idx(gidx)
            m[f"invrep_{br}"] = invrep[br]
            m[f"semq_{br}"] = np.ascontiguousarray(
                semT[:, core * QCH : (core + 1) * QCH])
            for k, vv in wts[br].items():
                m[f"{k}_{br}"] = vv
        in_maps.append(m)
    return in_maps
